# revision 62
# baseline (speedup 1.0000x reference)
"""Causal self-attention (B=4, S=2048, D=768, H=12) on 8 trn2 NeuronCores.

Sharding (Megatron-style): DP over the 4 batches x TP=2 over heads.
Core c handles batch c//2 with heads (c%2)*6 .. +6: qkv_proj column-parallel,
out_proj row-parallel; the TP pair's partial outputs are summed on the host.

Per-core kernel, fp16 data path (fp32 PSUM accumulation everywhere):
  A. x arrives host-transposed as xT [d, s] fp16; weights fp16, wqkv
     pre-swizzled so every f-tile DMA is one 1536B run per partition.  The
     score scale 1/sqrt(64) AND the fast-exp prescale 2^10*log2(e) are
     folded into Wq/bq on the host, so score PSUM holds s*1477.32.  The
     V-bias shifts every attention output by exactly qkv_b[2D:] (softmax
     weights sum to 1), so the host adds qkv_b[2D:] @ out_w to the result
     instead of the kernel touching it.
  B. qkT = (x @ Wqk)^T in [feat(part), s] layout (bias fused into the
     PSUM->SBUF drain); V in natural [s(part), feat] layout with a
     ones column (V') so PV also produces the softmax denominator.
  C. flash-style causal attention per (head, 512-q-chunk) job, k-tile by
     k-tile (4 single-bank score PSUM slots): S^T = K_tile @ Q^T
     (contraction 64).  Each tile's exp goes to whichever engine is
     modeled-free sooner (GPSIMD cannot read PSUM, so Pool only runs the
     causal-diagonal affine_select masks):
       - ACT: true exp via ACTIVATE (scale=1/1477.32, bias=-4ln2), or
       - DVE: Schraudolph fast-exp = ONE tensor_scalar (max,add) writing
         int16 whose bits ARE the fp16 exp(s-4ln2) (max rel err ~3%;
         measured end-to-end 8.7e-3 << 2e-2 even if used for ALL exps).
     Softmax numerator and denominator use the same p values, so the 2^-4
     scaling and the fast-exp error largely cancel in the ratio.  PV is
     FLIPPED: O[q(part), 65] += pt_tile^T @ V' - 65-wide moving operand.
     The denominator is per-PARTITION: DVE reciprocal + ONE broadcast
     tensor_tensor writes normalized O into o_pair [q, qt, h01*64+d] fp16;
     one XBAR DMA-transpose per (head-pair, q-chunk) then produces
     oT [feat(part), s] off-engine (PE transposes eliminated; the final
     block's late pairs PE-transpose instead to dodge the tail HWDGE queue).
  D. out_partial = O @ Wout_slice via lhsT=oT chunks, written [s, 768] fp16
     (the host sums the TP pair in fp32).

  Emission order is driven by a greedy scheduler: attention jobs emit
  k-tile by k-tile; PV flushes ride a global FIFO popped FLUSH_W-deep so
  each tile's exp has several tile-times to land cross-job before PE reads
  it.  Between tiles, PE "filler" work (qkv f-tiles, V s-tiles, out-proj
  units) is pulled as MICRO-STEPS (one matmul each) whenever modeled PE
  work falls behind the exp engines, so sub-us holes are packed tightly.
  PSUM->SBUF drains are engine-chosen (ACT activation vs DVE copy) by the
  same modeled clocks.  This keeps the Tensor engine continuously busy
  (and at the fast p-state) while ACT/DVE grind through the exps.
"""
from collections import deque
import math

import numpy as np
import concourse.bass as bass
import concourse.mybir as mybir
import concourse.tile as tile
from concourse import bacc
from concourse.bass_utils import run_bass_kernel_spmd
from concourse.masks import make_identity

B, S, D = 4, 2048, 768
H, HD = 12, 64
N_CORES = 8
HPC = H // 2          # heads per core = 6
FQK = HPC * HD        # 384 features per core for each of q,k,v
F32 = mybir.dt.float32
F16 = mybir.dt.float16
I16 = mybir.dt.int16

N_ST = S // 128       # 16 s tiles
N_QC = S // 512       # 4 q chunks
N_DT = D // 128       # 6 d_model tiles

PE_C = 1.0 / 2.4      # ns per PE row at full clock (cost model)
ACT_C = 1.0 / 1.2     # ns per ACT column
POOL_C = 1.0 / 1.2 / 0.6   # ns per Pool column (0.6 impl efficiency)

# Schraudolph fast-exp in fp16 bits: for prescaled u = s * A_EXP, the int16
# value (u max T_EXP) + B_EXP bitcast to fp16 is ~exp(s - 4ln2).  C_EXP
# tunes the truncation bias (scanned empirically).
A_EXP = 1024.0 * math.log2(math.e)          # 1477.3197
C_EXP = 55.0
B_EXP = 15.0 * 1024.0 - 4.0 * 1024.0 - C_EXP
T_EXP = 1.0 - B_EXP                          # clamp so v >= 1 (never <0)
EXP_BIAS = -4.0 * math.log(2.0)              # ACT path: exp(s + bias)

TRACE = False         # set by test.py for profiling runs
DEBUG = False
_CACHE = {}
PHASE_MARKS = []      # (phase_name, first_inst_id) - filled during _emit
EMIT_STATS = {}       # modeled clocks, for offline schedule debugging
STARVE_LOG = []       # (job, ns) filler-bank dry spells during emission
CUR_JOB = ["init"]

# job order: qc-clustered so each qc block COMPLETES as a unit and its
# out-projection units become filler for the next (heavier) block;
# head-pairs adjacent (shared o_pair tiles); light qc=0 block last so the
# serial end chain (normalize -> XBAR -> proj) is short
JOB_ORDER = [
    (0, 0), (1, 0),                                  # cheap-dep warmup
    (0, 1), (1, 1), (2, 1), (3, 1), (4, 1), (5, 1),
    (0, 3), (1, 3), (2, 3), (3, 3), (4, 3), (5, 3),
    (0, 2), (1, 2), (2, 2), (3, 2), (4, 2), (5, 2),
    (2, 0), (4, 0), (3, 0), (5, 0),
]


def _mark(nc, name):
    PHASE_MARKS.append((name, nc.next_id()))


def _emit(nc):
    xt_d = nc.dram_tensor("xt", [D, S], F16, kind="ExternalInput").ap()
    # wqkv pre-swizzled on host to [p, ftile, dtile, f] so each f-tile DMA
    # is one 1536B-contiguous run per partition (fast descriptors)
    wqkv_d = nc.dram_tensor("wqkv", [128, 9, N_DT, 128], F16,
                            kind="ExternalInput").ap()
    bqk_d = nc.dram_tensor("bqk", [128, 6], F32, kind="ExternalInput").ap()
    wout_d = nc.dram_tensor("wout", [FQK, D], F16, kind="ExternalInput").ap()
    out_d = nc.dram_tensor("out", [S, D], F16, kind="ExternalOutput").ap()

    with tile.TileContext(nc) as tc:
        with tc.tile_pool(name="const", bufs=1) as pc, \
             tc.tile_pool(name="xT", bufs=1) as pxt, \
             tc.tile_pool(name="qkT", bufs=1) as pqk, \
             tc.tile_pool(name="vn", bufs=1) as pvn, \
             tc.tile_pool(name="wq", bufs=1) as pwq, \
             tc.tile_pool(name="OT", bufs=1) as pot, \
             tc.tile_pool(name="pt", bufs=8) as ppt, \
             tc.tile_pool(name="opair", bufs=4) as pop, \
             tc.tile_pool(name="rc", bufs=8) as prc, \
             tc.tile_pool(name="outp", bufs=2) as pout, \
             tc.tile_pool(name="ps", bufs=4, space="PSUM") as pp, \
             tc.tile_pool(name="pso", bufs=2, space="PSUM") as ppo, \
             tc.tile_pool(name="aux", bufs=2, space="PSUM") as paux:

            bqk_sb = pc.tile([128, 6], F32)
            ident = pc.tile([128, 128], F16)
            make_identity(nc, ident[:])
            # zero operand for the PV group-opening matmul (one PSUM bank may
            # hold only ONE open accumulation group: interleaved per-region
            # start flags corrupt each other's partial sums on hardware)
            zeros = pc.tile([128, 4 * (HD + 1)], F16)
            nc.vector.memset(zeros[:], 0.0)
            expb = pc.tile([128, 1], F32)
            nc.vector.memset(expb[:], EXP_BIAS)

            xT = pxt.tile([128, N_DT, S], F16)
            qkT = pqk.tile([128, 6, S], F16)
            # Vn: [s(part), s_tile, head, 65] with ones col at 64
            vn = pvn.tile([128, N_ST, HPC, HD + 1], F16)
            wqkv_sb = pwq.tile([128, 9, N_DT, 128], F16)
            wout_sb = pwq.tile([128, FQK // 128, D], F16)
            # oT split [128, pair, st, 128] so the XBAR transpose's 3D output
            # AP is a plain slice
            oT = pot.tile([128, FQK // 128, N_ST, 128], F16)

            nc.vector.memset(vn[:, :, :, HD:HD + 1], 1.0)

            clk = {"pe": 0.0, "act": 0.0, "pool": 0.0, "dve": 0.0,
                   "hwdge": 0.0, "starve": 0.0,
                   "act_cols": 0.0, "pool_cols": 0.0, "dve_cols": 0.0}

            # ---- input DMAs, priority-ordered (HWDGE generates in order;
            # tile subtile-deps gate the first consumer of each slice) ----
            _mark(nc, "A:dma")

            def dma_w(ft):
                nc.sync.dma_start(wqkv_sb[:, ft, :, :], wqkv_d[:, ft, :, :])

            xt_r = xt_d.rearrange("(t p) s -> p t s", p=128)

            def dma_x(sc, split=False):
                if split:   # per-dc: consumers start sooner
                    for dc in range(N_DT):
                        nc.sync.dma_start(
                            xT[:, dc, sc * 512:(sc + 1) * 512],
                            xt_r[:, dc, sc * 512:(sc + 1) * 512])
                        if sc == 0 and dc == 0:
                            dma_w(3)   # k-chunk weights right after x dc0
                else:       # one HWDGE slot for the whole s-chunk
                    nc.sync.dma_start(xT[:, :, sc * 512:(sc + 1) * 512],
                                      xt_r[:, :, sc * 512:(sc + 1) * 512])
                clk["hwdge"] += (N_DT if split else 1) * 650.0

            # first jobs are (0,0),(1,0): need ft(0,0), ft(3,0), v0..3
            dma_w(0)
            dma_x(0, split=True)
            nc.sync.dma_start(bqk_sb[:], bqk_d[:])
            for g in range(6, 9):   # wv: the warmup jobs' PV flushes need it
                dma_w(g)
            dma_x(1)
            dma_w(1)
            dma_w(4)
            dma_x(2)
            dma_x(3)
            dma_w(2)
            dma_w(5)
            nc.sync.dma_start(
                wout_sb[:], wout_d.rearrange("(t p) o -> p t o", p=128))
            clk["hwdge"] += 16 * 650.0   # w/bqk/vb/wout slots

            # ---- virtual-clock list scheduler: pe/act/pool track the
            # modeled finish time of issued work per engine.  Before a PV
            # flush (which needs its group's exp done), fillers are emitted
            # until PE's frontier covers the exp-ready time, so PE never
            # idles waiting on the exp engines. ----
            def dve_work(cols):
                # DVE drains/normalizes trail emission order; model them so
                # the exp chooser sees DVE's true backlog
                clk["dve"] = max(clk["dve"], clk["pe"]) + cols * 1.05 + 150.0

            def pick_drain(cols):
                """Pick the engine for a PSUM->SBUF drain by modeled clocks."""
                start = clk["pe"] + 150.0
                fins = {
                    "act": max(clk["act"], start) + cols * ACT_C + 370.0,
                    "dve": max(clk["dve"], start) + cols * 1.05 + 260.0,
                }
                eng = min(fins, key=lambda e: fins[e])
                clk[eng] = fins[eng]
                return eng

            def drain_copy(out_ap, in_ap, cols):
                eng = pick_drain(cols)
                if eng == "act":
                    nc.scalar.copy(out_ap, in_ap)
                else:
                    nc.vector.tensor_copy(out_ap, in_ap)

            def drain_bias_add(out_ap, in_ap, bias_col, cols):
                eng = pick_drain(cols)
                if eng == "act":
                    nc.scalar.activation(
                        out_ap, in_ap,
                        mybir.ActivationFunctionType.Identity, bias=bias_col)
                else:
                    nc.vector.tensor_scalar(out_ap, in_ap, bias_col, None,
                                            mybir.AluOpType.add)
            fillers = deque()     # keys, FIFO
            filler_fns = {}       # key -> (pe_cost, fn)
            filler_ready = {}     # key -> earliest clk.pe this unit can run
            emitted = set()
            opair_store = {}      # (pair, qc) -> o_pair sbuf tile

            def register(key, steps, ready=0.0):
                """steps: list of (pe_cost, fn) micro-steps.  fill_until
                executes one micro-step at a time so PE waits are packed
                tightly (a 1.3us unit poured into a 0.4us hole would
                otherwise delay the score stream by the overshoot)."""
                filler_fns[key] = deque(steps)
                filler_ready[key] = ready
                fillers.append(key)

            open_unit = [None]   # the (at most one) partially-filled unit

            def force(key):
                if key in emitted:
                    return
                # a partially-filled unit holds an aux slot; finish it first
                # so concurrent aux tenants never exceed the ring size
                if open_unit[0] is not None and open_unit[0] != key:
                    prev, open_unit[0] = open_unit[0], None
                    force(prev)
                if open_unit[0] == key:
                    open_unit[0] = None
                emitted.add(key)
                steps = filler_fns[key]
                while steps:
                    cost, fn = steps.popleft()
                    fn()
                    clk["pe"] += cost

            def fill_until(t):
                if clk.get("filling"):
                    return
                clk["filling"] = True
                try:
                    _fill_until(t)
                finally:
                    clk["filling"] = False

            def _fill_until(t):
                scanned = 0
                while fillers and clk["pe"] < t and scanned < len(fillers):
                    key = fillers[0]
                    if key in emitted:
                        fillers.popleft()
                        continue
                    rdy = filler_ready.get(key, 0.0)
                    if rdy > clk["pe"]:
                        if rdy < t and scanned + 1 >= len(fillers):
                            # would starve anyway: jump to its ready time
                            clk["pe"] = rdy
                            continue
                        fillers.rotate(-1)   # not ready yet; rotate
                        scanned += 1
                        continue
                    scanned = 0
                    steps = filler_fns[key]
                    cost, fn = steps.popleft()
                    fn()
                    clk["pe"] += cost
                    if not steps:
                        emitted.add(key)
                        fillers.popleft()
                        open_unit[0] = None
                    else:
                        open_unit[0] = key
                if clk["pe"] < t:
                    clk["starve"] += t - clk["pe"]
                    STARVE_LOG.append((CUR_JOB[0], round(t - clk["pe"]),
                                       [k for k in fillers
                                        if k not in emitted]))

            # PSUM slots pace emission: allocating one waits (in hardware)
            # for its previous tenant's consumer to drain, so model each
            # slot ring's free times and pour filler into score-slot waits.
            # Scores own the "s" ring; filler/proj units share the 2-slot
            # "aux" ring so they never stall the score stream.
            s_free = deque([0.0, 0.0, 0.0, 0.0])
            aux_free = deque([0.0, 0.0])

            def s_gate():
                gate = s_free.popleft()
                fill_until(gate)
                clk["pe"] = max(clk["pe"], gate)

            def aux_gate():
                gate = aux_free.popleft()
                clk["pe"] = max(clk["pe"], gate)

            # ---- filler units as micro-step lists ----
            def v_steps(st):
                cell = {}

                def step(dc):
                    if dc == 0:
                        aux_gate()
                        cell["ps"] = paux.tile([128, FQK], F32, tag="aux", name="ps_v")
                    nc.tensor.matmul(
                        cell["ps"][:, :],
                        xT[:, dc, st * 128:(st + 1) * 128],
                        wqkv_sb[:, 6:9, dc, :],
                        start=(dc == 0), stop=(dc == N_DT - 1))
                    if dc == N_DT - 1:
                        drain_copy(
                            vn[:, st, :, 0:HD],
                            cell["ps"][:, :].rearrange(
                                "p (h d) -> p h d", d=HD), FQK)
                        aux_free.append(clk["pe"] + FQK * PE_C + 800.0)
                return [(FQK * PE_C, lambda d=dc: step(d))
                        for dc in range(N_DT)]

            def ft_steps(ft, sc):
                cell = {}

                def step(dc):
                    if dc == 0:
                        aux_gate()
                        cell["ps"] = paux.tile([128, 512], F32, tag="aux", name="ps_qk")
                    nc.tensor.matmul(
                        cell["ps"][:, :],
                        wqkv_sb[:, ft, dc, :],
                        xT[:, dc, sc * 512:(sc + 1) * 512],
                        start=(dc == 0), stop=(dc == N_DT - 1))
                    if dc == N_DT - 1:
                        drain_bias_add(qkT[:, ft, sc * 512:(sc + 1) * 512],
                                       cell["ps"][:, :],
                                       bqk_sb[:, ft:ft + 1], 512)
                        aux_free.append(clk["pe"] + 512 * PE_C + 800.0)
                return [(512 * PE_C, lambda d=dc: step(d))
                        for dc in range(N_DT)]

            proj_osb = {}

            def proj_steps(st, tail=False):
                """Both 384-col halves of one s-tile's projection as a
                single 6-step unit (order within the unit is guaranteed; the
                oc=1 drain ships the whole row)."""
                cell = {}

                def step(oc, ht):
                    if oc == 0 and ht == 0:
                        proj_osb[st] = pout.tile([128, D], F16,
                                                 tag="osb", name="o_sb")
                    if ht == 0:
                        if tail:
                            ps_big = pp.tile([128, 512], F32, tag="s",
                                             name="ps_big")
                            cell["ps"] = ps_big[:, 0:384]
                        else:
                            aux_gate()
                            cell["ps"] = paux.tile([128, 384], F32,
                                                   tag="aux", name="ps_d")
                    nc.tensor.matmul(
                        cell["ps"][:, :],
                        oT[:, ht, st, :],
                        wout_sb[:, ht, oc * 384:(oc + 1) * 384],
                        start=(ht == 0), stop=(ht == FQK // 128 - 1))
                    if ht == FQK // 128 - 1:
                        o_sb = proj_osb[st]
                        drain_copy(o_sb[:, oc * 384:(oc + 1) * 384],
                                   cell["ps"][:, :], 384)
                        if not tail:
                            aux_free.append(clk["pe"] + 384 * PE_C + 700.0)
                        if oc == 1:
                            nc.sync.dma_start(
                                out_d[st * 128:(st + 1) * 128, :], o_sb[:])
                            clk["hwdge"] = (max(clk["hwdge"], clk["pe"])
                                            + 650.0)
                            del proj_osb[st]

                return [(384 * PE_C, lambda o=oc, t=ht: step(o, t))
                        for oc in range(2) for ht in range(FQK // 128)]

            def emit_proj(st, tail=False):
                for cost, fn in proj_steps(st, tail):
                    fn()
                    clk["pe"] += cost

            for _ft in range(6):
                for _sc in range(4):
                    register(("ft", _ft, _sc), ft_steps(_ft, _sc))
            for _st in range(N_ST):
                register(("v", _st), v_steps(_st))

            # initial queue order: first-use-ish; later heads' weight tiles
            # and V tiles trail so they stay banked for the ACT-heavy midgame
            fillers.clear()
            for key in ([("ft", 0, 0), ("ft", 3, 0), ("v", 0), ("v", 1),
                         ("ft", 0, 1), ("ft", 3, 1), ("v", 2), ("v", 3)]
                        + [("ft", f, s) for s in range(2)
                           for f in (1, 4, 2, 5)]
                        + [("v", _st) for _st in range(4, 10)]
                        + [("ft", f, s) for s in range(2, 4)
                           for f in (0, 3, 1, 4, 2, 5)]
                        + [("v", _st) for _st in range(10, 16)]):
                fillers.append(key)

            def prioritize(key):
                if key in emitted or key not in filler_fns:
                    return
                try:
                    fillers.remove(key)
                except ValueError:
                    pass
                fillers.appendleft(key)

            # ---- attention job: k-loop with a cross-job PV-deferral
            # window: flushes ride a global FIFO, popped W-deep so each
            # k-tile's exp has ~W tile-times to land before PE reads it ----
            FLUSH_W = 5
            flushq = deque()      # (job_idx, flush_closure)
            normed = {qc: 0 for qc in range(N_QC)}

            def note_normed(h, qc, last_block=False):
                normed[qc] += 1
                if h % 2 == 1:
                    # both heads of the pair normalized: transpose the shared
                    # o_pair tile into oT.  Normally via XBAR DMA (off-PE);
                    # for the final block's late pairs the HWDGE round-trip
                    # would sit on the critical tail, so PE-transpose instead.
                    pair = h // 2
                    op_t = opair_store.pop((pair, qc))
                    if last_block and pair >= 1:
                        aux_gate()
                        ps_tr = paux.tile([128, 512], F16, tag="aux",
                                          name="ps_tr")
                        for qt in range(4):
                            for hh in range(2):
                                nc.tensor.transpose(
                                    ps_tr[hh * 64:(hh + 1) * 64,
                                          qt * 128:(qt + 1) * 128],
                                    op_t[:, qt, hh * 64:(hh + 1) * 64],
                                    ident[:])
                        clk["pe"] += 8 * 128 * PE_C
                        for sti in range(4):
                            drain_copy(oT[:, pair, qc * 4 + sti, :],
                                       ps_tr[:, sti * 128:(sti + 1) * 128],
                                       128)
                        aux_free.append(clk["pe"] + 8 * 128 * PE_C + 800.0)
                    else:
                        nc.sync.dma_start_transpose(
                            oT[:, pair, qc * 4:(qc + 1) * 4, :], op_t[:])
                        clk["hwdge"] = max(clk["hwdge"], clk["pe"]) + 650.0
                if normed[qc] == HPC:
                    # proj waits the XBARs through the serial HWDGE queue
                    rdy = max(clk["pe"] + 1500.0, clk["hwdge"] + 1200.0)
                    if last_block:
                        rdy = clk["pe"] + 600.0
                    for st in range(qc * 4, qc * 4 + 4):
                        register(("proj", st), proj_steps(st), ready=rdy)

            def force_deps(h, qc):
                # q f_tile chunk qc; k chunks are forced JIT in the k-loop
                force(("ft", h // 2, qc))
                force(("ft", 3 + h // 2, 0))

            def emit_exp(ps_s, pt, offs, diag_js):
                """Exp one 2-k-tile pair on ACT (true exp) or DVE (fast-exp),
                whichever is modeled-free sooner (GPSIMD cannot read PSUM);
                diagonal tiles get a causal mask on Pool.  Returns the
                flush-ready time."""
                if offs[0] >= 256:
                    spans = [(j * 512 + offs[j], (j + 1) * 512)
                             for j in range(2)]
                else:
                    spans = [(offs[0], 1024)]
                cols = sum(b - a for a, b in spans)
                n = len(spans)
                start = clk["pe"] + 150.0
                fins = {
                    "act": max(clk["act"], start) + cols * ACT_C + n * 370.0,
                    "dve": max(clk["dve"], start) + cols * 1.05 + n * 400.0,
                }
                eng = min(fins, key=lambda e: fins[e])
                if eng == "act":
                    for a, b in spans:
                        nc.scalar.activation(
                            pt[:, a:b], ps_s[:, a:b],
                            mybir.ActivationFunctionType.Exp,
                            bias=expb[:], scale=1.0 / A_EXP)
                else:
                    for a, b in spans:
                        nc.vector.tensor_scalar(
                            pt[:, a:b].bitcast(I16), ps_s[:, a:b],
                            T_EXP, B_EXP,
                            mybir.AluOpType.max, mybir.AluOpType.add)
                clk[eng] = fins[eng]
                clk[eng + "_cols"] += cols
                fin = fins[eng]
                for j in diag_js:
                    # causal mask, emitted right after the exp so Pool
                    # applies it long before the deferred PV flush
                    q_off = offs[j]
                    sl = slice(j * 512 + q_off, j * 512 + q_off + 128)
                    nc.gpsimd.affine_select(
                        out=pt[:, sl], in_=pt[:, sl],
                        compare_op=mybir.AluOpType.is_ge,
                        fill=0.0, base=0, channel_multiplier=-1,
                        pattern=[[1, 128]])
                    clk["pool"] = (max(clk["pool"], fin + 100.0)
                                   + 128 * POOL_C + 100.0)
                    fin = clk["pool"]
                return fin + 150.0

            def emit_exp1(ps_s, pt, q_off, diag):
                cols = 512 - q_off
                start = clk["pe"] + 150.0
                fins = {
                    "act": max(clk["act"], start) + cols * ACT_C + 370.0,
                    "dve": max(clk["dve"], start) + cols * 1.05 + 400.0,
                }
                eng = min(fins, key=lambda e: fins[e])
                if eng == "act":
                    nc.scalar.activation(
                        pt[:, q_off:], ps_s[:, q_off:],
                        mybir.ActivationFunctionType.Exp,
                        bias=expb[:], scale=1.0 / A_EXP)
                else:
                    nc.vector.tensor_scalar(
                        pt[:, q_off:].bitcast(I16), ps_s[:, q_off:],
                        T_EXP, B_EXP,
                        mybir.AluOpType.max, mybir.AluOpType.add)
                clk[eng] = fins[eng]
                clk[eng + "_cols"] += cols
                fin = fins[eng]
                if diag:
                    nc.gpsimd.affine_select(
                        out=pt[:, q_off:q_off + 128],
                        in_=pt[:, q_off:q_off + 128],
                        compare_op=mybir.AluOpType.is_ge,
                        fill=0.0, base=0, channel_multiplier=-1,
                        pattern=[[1, 128]])
                    clk["pool"] = (max(clk["pool"], fin + 100.0)
                                   + 128 * POOL_C + 100.0)
                    fin = clk["pool"]
                return fin + 150.0

            def attn_job(job_idx, h, qc):
                po = (h % 2) * 64
                qf = h // 2
                kf = 3 + h // 2
                # the ps_o ring has 2 slots: job i-2's accumulation (incl.
                # normalize) must fully drain before this job's open
                while flushq and flushq[0][0] <= job_idx - 2:
                    flushq.popleft()[1]()
                force_deps(h, qc)
                ps_o = ppo.tile([128, 4, HD + 1], F32, tag="o")
                n_kt = 4 * (qc + 1)
                # open ONE accumulation group for the whole bank, zeroing all
                # four qt regions; every PV then accumulates with start=False
                nc.tensor.matmul(ps_o[:, :, :], ident[:], zeros[:],
                                 start=True, stop=False)
                clk["pe"] += 4 * (HD + 1) * PE_C

                def flush(kt, pt, base, ready, last):
                    fill_until(ready)
                    clk["pe"] = max(clk["pe"], ready)
                    force(("v", kt))
                    for qt in range(4):
                        if kt <= qc * 4 + qt:
                            nc.tensor.matmul(
                                ps_o[:, qt, :],
                                pt[:, base + qt * 128:base + (qt + 1) * 128],
                                vn[:, kt, h, :],
                                start=False,
                                stop=(kt == n_kt - 1 and qt == 3))
                            clk["pe"] += 65 * PE_C
                    if last:
                        recip = prc.tile([128, 4, 1], F32, tag="rc")
                        nc.vector.reciprocal(recip[:], ps_o[:, :, HD:HD + 1])
                        pair = h // 2
                        if (pair, qc) not in opair_store:
                            opair_store[(pair, qc)] = pop.tile(
                                [128, 4, 128], F16, tag="opair", name="o_pair")
                        op_t = opair_store[(pair, qc)]
                        # one batched normalize: recip broadcast along d
                        nc.vector.tensor_tensor(
                            op_t[:, :, po:po + HD], ps_o[:, :, 0:HD],
                            recip[:].broadcast_to([128, 4, HD]),
                            mybir.AluOpType.mult)
                        dve_work(4 * HD)
                        note_normed(h, qc, last_block=(qc == 0 and h >= 2))

                for kt in range(n_kt):
                    force(("ft", kf, kt // 4))
                    s_gate()
                    ps_s = pp.tile([128, 512], F32, tag="s")
                    pt = ppt.tile([128, 512], F16, tag="pt")
                    q_off = max(0, kt * 128 - qc * 512)
                    diag = kt * 128 >= qc * 512
                    nc.tensor.matmul(
                        ps_s[:, q_off:],
                        qkT[po:po + 64, kf, kt * 128:(kt + 1) * 128],
                        qkT[po:po + 64, qf,
                            qc * 512 + q_off:(qc + 1) * 512],
                        start=True, stop=True)
                    clk["pe"] += (512 - q_off) * PE_C
                    ready = emit_exp1(ps_s, pt, q_off, diag)
                    s_free.append(ready - 150.0)
                    flushq.append(
                        (job_idx, lambda k=kt, p=pt, r=ready,
                         l=(kt == n_kt - 1): flush(k, p, 0, r, l)))
                    while len(flushq) > FLUSH_W:
                        flushq.popleft()[1]()

            for i, (h, qc) in enumerate(JOB_ORDER):
                _mark(nc, f"C:h{h} qc{qc}")
                CUR_JOB[0] = f"h{h}q{qc}"
                if i + 1 < len(JOB_ORDER):
                    # float the next job's dep tiles to the queue front so
                    # they get pulled as filler during THIS job instead of
                    # landing as an ACT-stalling blob at the boundary
                    nh, nqc = JOB_ORDER[i + 1]
                    for key in [("ft", 3 + nh // 2, sc)
                                for sc in range(nqc, -1, -1)] + \
                               [("ft", nh // 2, nqc)]:
                        prioritize(key)
                attn_job(i, h, qc)

            _mark(nc, "D:tail")
            CUR_JOB[0] = "tail"
            while flushq:   # final jobs' deferred PVs + normalizes
                flushq.popleft()[1]()
            if open_unit[0] is not None:
                force(open_unit[0])
            # drain every remaining filler; projs not yet emitted use the
            # (now free) score-psum banks so halves never serialize
            for st in range(N_ST):
                key = ("proj", st)
                if key in filler_fns and key not in emitted:
                    emitted.add(key)
                    emit_proj(st, tail=True)
            # anything else (ft/v) must already be in; assert coverage
            for key in filler_fns:
                assert key in emitted, f"filler {key} never emitted"
            assert not opair_store, f"unfinished o_pair {list(opair_store)}"
            EMIT_STATS.update(clk)


def _build():
    if "nc" not in _CACHE:
        nc = bacc.Bacc("TRN2", target_bir_lowering=False, debug=False,
                       num_devices=N_CORES)
        _emit(nc)
        nc.compile()
        _CACHE["nc"] = nc
    return _CACHE["nc"]


def kernel(x, qkv_w, qkv_b, out_w, out_b):
    x = np.asarray(x, dtype=np.float32)
    qkv_w = np.asarray(qkv_w, dtype=np.float32)
    qkv_b = np.asarray(qkv_b, dtype=np.float32)
    out_w = np.asarray(out_w, dtype=np.float32)
    out_b = np.asarray(out_b, dtype=np.float32)

    nc = _build()
    scale = HD ** -0.5 * A_EXP   # score scale + fast-exp prescale
    in_maps = []
    for c in range(N_CORES):
        b, half = c // 2, c % 2
        fq = slice(half * FQK, (half + 1) * FQK)
        fk = slice(D + half * FQK, D + (half + 1) * FQK)
        fv = slice(2 * D + half * FQK, 2 * D + (half + 1) * FQK)
        wq = qkv_w[:, fq] * scale
        wk = qkv_w[:, fk]
        wv = qkv_w[:, fv]
        wqkv = np.concatenate([wq, wk, wv], axis=1)   # [768, 1152]
        # swizzle to [p, ftile, dtile, f] (one contiguous 1536B run per
        # partition per f-tile DMA)
        wqkv = np.ascontiguousarray(
            wqkv.reshape(N_DT, 128, 9, 128).transpose(1, 2, 0, 3),
            dtype=np.float16)
        bqk = np.concatenate([qkv_b[fq] * scale, qkv_b[fk]])  # [768]
        bqk = np.ascontiguousarray(
            bqk.reshape(6, 128).T, dtype=np.float32)          # [128, 6]
        wout = np.ascontiguousarray(
            out_w[half * FQK:(half + 1) * FQK, :], dtype=np.float16)
        in_maps.append({
            "xt": np.ascontiguousarray(x[b].T).astype(np.float16),
            "wqkv": wqkv, "bqk": bqk, "wout": wout,
        })

    res = run_bass_kernel_spmd(nc, in_maps, list(range(N_CORES)), trace=TRACE)
    parts = [res.results[c]["out"] for c in range(N_CORES)]
    # the V bias shifts every attention output by exactly qkv_b[2D:]
    # (softmax weights sum to 1), so its effect on the final output is the
    # constant row qkv_b[2D:] @ out_w - applied here instead of on-core
    bias_row = out_b + qkv_b[2 * D:] @ out_w
    out = np.empty((B, S, D), dtype=np.float32)
    for b in range(B):
        out[b] = (parts[2 * b].astype(np.float32)
                  + parts[2 * b + 1].astype(np.float32) + bias_row)
    if TRACE:
        kernel.last_results = res
    return out


# revision 63
# speedup vs baseline: 1.0083x; 1.0083x over previous
"""Causal self-attention (B=4, S=2048, D=768, H=12) on 8 trn2 NeuronCores.

Sharding (Megatron-style): DP over the 4 batches x TP=2 over heads.
Core c handles batch c//2 with heads (c%2)*6 .. +6: qkv_proj column-parallel,
out_proj row-parallel; the TP pair's partial outputs are summed on the host.

Per-core kernel, fp16 data path (fp32 PSUM accumulation everywhere):
  A. x arrives host-transposed as xT [d, s] fp16; weights fp16, wqkv
     pre-swizzled so every f-tile DMA is one 1536B run per partition.  The
     score scale 1/sqrt(64) AND the fast-exp prescale 2^10*log2(e) are
     folded into Wq/bq on the host, so score PSUM holds s*1477.32.  The
     V-bias shifts every attention output by exactly qkv_b[2D:] (softmax
     weights sum to 1), so the host adds qkv_b[2D:] @ out_w to the result
     instead of the kernel touching it.
  B. qkT = (x @ Wqk)^T in [feat(part), s] layout (bias fused into the
     PSUM->SBUF drain); V in natural [s(part), feat] layout with a
     ones column (V') so PV also produces the softmax denominator.
  C. flash-style causal attention per (head, 512-q-chunk) job, k-tile by
     k-tile (4 single-bank score PSUM slots): S^T = K_tile @ Q^T
     (contraction 64).  Each tile's exp goes to whichever engine is
     modeled-free sooner (GPSIMD cannot read PSUM, so Pool only runs the
     causal-diagonal affine_select masks):
       - ACT: true exp via ACTIVATE (scale=1/1477.32, bias=-4ln2), or
       - DVE: Schraudolph fast-exp = ONE tensor_scalar (max,add) writing
         int16 whose bits ARE the fp16 exp(s-4ln2) (max rel err ~3%;
         measured end-to-end 8.7e-3 << 2e-2 even if used for ALL exps).
     Softmax numerator and denominator use the same p values, so the 2^-4
     scaling and the fast-exp error largely cancel in the ratio.  PV is
     FLIPPED: O[q(part), 65] += pt_tile^T @ V' - 65-wide moving operand.
     The denominator is per-PARTITION: DVE reciprocal + ONE broadcast
     tensor_tensor writes normalized O into o_pair [q, qt, h01*64+d] fp16;
     one XBAR DMA-transpose per (head-pair, q-chunk) then produces
     oT [feat(part), s] off-engine (PE transposes eliminated; the final
     block's late pairs PE-transpose instead to dodge the tail HWDGE queue).
  D. out_partial = O @ Wout_slice via lhsT=oT chunks, written [s, 768] fp16
     (the host sums the TP pair in fp32).

  Emission order is driven by a greedy scheduler: attention jobs emit
  k-tile by k-tile; PV flushes ride a global FIFO popped FLUSH_W-deep so
  each tile's exp has several tile-times to land cross-job before PE reads
  it.  Between tiles, PE "filler" work (qkv f-tiles, V s-tiles, out-proj
  units) is pulled as MICRO-STEPS (one matmul each) whenever modeled PE
  work falls behind the exp engines, so sub-us holes are packed tightly.
  PSUM->SBUF drains are engine-chosen (ACT activation vs DVE copy) by the
  same modeled clocks.  This keeps the Tensor engine continuously busy
  (and at the fast p-state) while ACT/DVE grind through the exps.
"""
from collections import deque
import math

import numpy as np
import concourse.bass as bass
import concourse.mybir as mybir
import concourse.tile as tile
from concourse import bacc
from concourse.bass_utils import run_bass_kernel_spmd
from concourse.masks import make_identity

B, S, D = 4, 2048, 768
H, HD = 12, 64
N_CORES = 8
HPC = H // 2          # heads per core = 6
FQK = HPC * HD        # 384 features per core for each of q,k,v
F32 = mybir.dt.float32
F16 = mybir.dt.float16
I16 = mybir.dt.int16

N_ST = S // 128       # 16 s tiles
N_QC = S // 512       # 4 q chunks
N_DT = D // 128       # 6 d_model tiles

PE_C = 1.0 / 2.4      # ns per PE row at full clock (cost model)
ACT_C = 1.0 / 1.2     # ns per ACT column
POOL_C = 1.0 / 1.2 / 0.6   # ns per Pool column (0.6 impl efficiency)

# Schraudolph fast-exp in fp16 bits: for prescaled u = s * A_EXP, the int16
# value (u max T_EXP) + B_EXP bitcast to fp16 is ~exp(s - 4ln2).  C_EXP
# tunes the truncation bias (scanned empirically).
A_EXP = 1024.0 * math.log2(math.e)          # 1477.3197
C_EXP = 55.0
B_EXP = 15.0 * 1024.0 - 4.0 * 1024.0 - C_EXP
T_EXP = 1.0 - B_EXP                          # clamp so v >= 1 (never <0)
EXP_BIAS = -4.0 * math.log(2.0)              # ACT path: exp(s + bias)

TRACE = False         # set by test.py for profiling runs
DEBUG = False
_CACHE = {}
PHASE_MARKS = []      # (phase_name, first_inst_id) - filled during _emit
EMIT_STATS = {}       # modeled clocks, for offline schedule debugging
STARVE_LOG = []       # (job, ns) filler-bank dry spells during emission
CUR_JOB = ["init"]

# job order: qc-clustered so each qc block COMPLETES as a unit and its
# out-projection units become filler for the next (heavier) block;
# head-pairs adjacent (shared o_pair tiles); light qc=0 block last so the
# serial end chain (normalize -> XBAR -> proj) is short
JOB_ORDER = [
    (0, 0), (1, 0),                                  # cheap-dep warmup
    (0, 1), (1, 1), (2, 1), (3, 1), (4, 1), (5, 1),
    (0, 2), (1, 2), (2, 2), (3, 2), (4, 2), (5, 2),
    (0, 3), (1, 3), (2, 3), (3, 3), (4, 3), (5, 3),
    (2, 0), (4, 0), (3, 0), (5, 0),
]


def _mark(nc, name):
    PHASE_MARKS.append((name, nc.next_id()))


def _emit(nc):
    xt_d = nc.dram_tensor("xt", [D, S], F16, kind="ExternalInput").ap()
    # wqkv pre-swizzled on host to [p, ftile, dtile, f] so each f-tile DMA
    # is one 1536B-contiguous run per partition (fast descriptors)
    wqkv_d = nc.dram_tensor("wqkv", [128, 9, N_DT, 128], F16,
                            kind="ExternalInput").ap()
    bqk_d = nc.dram_tensor("bqk", [128, 6], F32, kind="ExternalInput").ap()
    wout_d = nc.dram_tensor("wout", [FQK, D], F16, kind="ExternalInput").ap()
    out_d = nc.dram_tensor("out", [S, D], F16, kind="ExternalOutput").ap()

    with tile.TileContext(nc) as tc:
        with tc.tile_pool(name="const", bufs=1) as pc, \
             tc.tile_pool(name="xT", bufs=1) as pxt, \
             tc.tile_pool(name="qkT", bufs=1) as pqk, \
             tc.tile_pool(name="vn", bufs=1) as pvn, \
             tc.tile_pool(name="wq", bufs=1) as pwq, \
             tc.tile_pool(name="OT", bufs=1) as pot, \
             tc.tile_pool(name="pt", bufs=8) as ppt, \
             tc.tile_pool(name="opair", bufs=4) as pop, \
             tc.tile_pool(name="rc", bufs=8) as prc, \
             tc.tile_pool(name="outp", bufs=2) as pout, \
             tc.tile_pool(name="ps", bufs=4, space="PSUM") as pp, \
             tc.tile_pool(name="pso", bufs=2, space="PSUM") as ppo, \
             tc.tile_pool(name="aux", bufs=2, space="PSUM") as paux:

            bqk_sb = pc.tile([128, 6], F32)
            ident = pc.tile([128, 128], F16)
            make_identity(nc, ident[:])
            # zero operand for the PV group-opening matmul (one PSUM bank may
            # hold only ONE open accumulation group: interleaved per-region
            # start flags corrupt each other's partial sums on hardware)
            zeros = pc.tile([128, 4 * (HD + 1)], F16)
            nc.vector.memset(zeros[:], 0.0)
            expb = pc.tile([128, 1], F32)
            nc.vector.memset(expb[:], EXP_BIAS)

            xT = pxt.tile([128, N_DT, S], F16)
            qkT = pqk.tile([128, 6, S], F16)
            # Vn: [s(part), s_tile, head, 65] with ones col at 64
            vn = pvn.tile([128, N_ST, HPC, HD + 1], F16)
            wqkv_sb = pwq.tile([128, 9, N_DT, 128], F16)
            wout_sb = pwq.tile([128, FQK // 128, D], F16)
            # oT split [128, pair, st, 128] so the XBAR transpose's 3D output
            # AP is a plain slice
            oT = pot.tile([128, FQK // 128, N_ST, 128], F16)

            nc.vector.memset(vn[:, :, :, HD:HD + 1], 1.0)

            clk = {"pe": 0.0, "act": 0.0, "pool": 0.0, "dve": 0.0,
                   "hwdge": 0.0, "starve": 0.0,
                   "act_cols": 0.0, "pool_cols": 0.0, "dve_cols": 0.0}

            # ---- input DMAs, priority-ordered (HWDGE generates in order;
            # tile subtile-deps gate the first consumer of each slice) ----
            _mark(nc, "A:dma")

            def dma_w(ft):
                nc.sync.dma_start(wqkv_sb[:, ft, :, :], wqkv_d[:, ft, :, :])

            xt_r = xt_d.rearrange("(t p) s -> p t s", p=128)

            def dma_x(sc, split=False):
                if split:   # per-dc: consumers start sooner
                    for dc in range(N_DT):
                        nc.sync.dma_start(
                            xT[:, dc, sc * 512:(sc + 1) * 512],
                            xt_r[:, dc, sc * 512:(sc + 1) * 512])
                        if sc == 0 and dc == 0:
                            dma_w(3)   # k-chunk weights right after x dc0
                else:       # one HWDGE slot for the whole s-chunk
                    nc.sync.dma_start(xT[:, :, sc * 512:(sc + 1) * 512],
                                      xt_r[:, :, sc * 512:(sc + 1) * 512])
                clk["hwdge"] += (N_DT if split else 1) * 650.0

            # first jobs are (0,0),(1,0): need ft(0,0), ft(3,0), v0..3
            dma_w(0)
            dma_x(0, split=True)
            nc.sync.dma_start(bqk_sb[:], bqk_d[:])
            dma_x(1)
            for g in range(6, 9):   # wv: first V units need it early
                dma_w(g)
            dma_w(1)
            dma_w(4)
            dma_x(2)
            dma_x(3)
            dma_w(2)
            dma_w(5)
            nc.sync.dma_start(
                wout_sb[:], wout_d.rearrange("(t p) o -> p t o", p=128))
            clk["hwdge"] += 16 * 650.0   # w/bqk/vb/wout slots

            # ---- virtual-clock list scheduler: pe/act/pool track the
            # modeled finish time of issued work per engine.  Before a PV
            # flush (which needs its group's exp done), fillers are emitted
            # until PE's frontier covers the exp-ready time, so PE never
            # idles waiting on the exp engines. ----
            def dve_work(cols):
                # DVE drains/normalizes trail emission order; model them so
                # the exp chooser sees DVE's true backlog
                clk["dve"] = max(clk["dve"], clk["pe"]) + cols * 1.05 + 150.0

            def pick_drain(cols):
                """Pick the engine for a PSUM->SBUF drain by modeled clocks."""
                start = clk["pe"] + 150.0
                fins = {
                    "act": max(clk["act"], start) + cols * ACT_C + 370.0,
                    "dve": max(clk["dve"], start) + cols * 1.05 + 260.0,
                }
                eng = min(fins, key=lambda e: fins[e])
                clk[eng] = fins[eng]
                return eng

            def drain_copy(out_ap, in_ap, cols):
                eng = pick_drain(cols)
                if eng == "act":
                    nc.scalar.copy(out_ap, in_ap)
                else:
                    nc.vector.tensor_copy(out_ap, in_ap)

            def drain_bias_add(out_ap, in_ap, bias_col, cols):
                eng = pick_drain(cols)
                if eng == "act":
                    nc.scalar.activation(
                        out_ap, in_ap,
                        mybir.ActivationFunctionType.Identity, bias=bias_col)
                else:
                    nc.vector.tensor_scalar(out_ap, in_ap, bias_col, None,
                                            mybir.AluOpType.add)
            fillers = deque()     # keys, FIFO
            filler_fns = {}       # key -> (pe_cost, fn)
            filler_ready = {}     # key -> earliest clk.pe this unit can run
            emitted = set()
            opair_store = {}      # (pair, qc) -> o_pair sbuf tile

            def register(key, steps, ready=0.0):
                """steps: list of (pe_cost, fn) micro-steps.  fill_until
                executes one micro-step at a time so PE waits are packed
                tightly (a 1.3us unit poured into a 0.4us hole would
                otherwise delay the score stream by the overshoot)."""
                filler_fns[key] = deque(steps)
                filler_ready[key] = ready
                fillers.append(key)

            open_unit = [None]   # the (at most one) partially-filled unit

            def force(key):
                if key in emitted:
                    return
                # a partially-filled unit holds an aux slot; finish it first
                # so concurrent aux tenants never exceed the ring size
                if open_unit[0] is not None and open_unit[0] != key:
                    prev, open_unit[0] = open_unit[0], None
                    force(prev)
                if open_unit[0] == key:
                    open_unit[0] = None
                emitted.add(key)
                steps = filler_fns[key]
                while steps:
                    cost, fn = steps.popleft()
                    fn()
                    clk["pe"] += cost

            def fill_until(t):
                if clk.get("filling"):
                    return
                clk["filling"] = True
                try:
                    _fill_until(t)
                finally:
                    clk["filling"] = False

            def _fill_until(t):
                scanned = 0
                while fillers and clk["pe"] < t and scanned < len(fillers):
                    key = fillers[0]
                    if key in emitted:
                        fillers.popleft()
                        continue
                    rdy = filler_ready.get(key, 0.0)
                    if rdy > clk["pe"]:
                        if rdy < t and scanned + 1 >= len(fillers):
                            # would starve anyway: jump to its ready time
                            clk["pe"] = rdy
                            continue
                        fillers.rotate(-1)   # not ready yet; rotate
                        scanned += 1
                        continue
                    scanned = 0
                    steps = filler_fns[key]
                    cost, fn = steps.popleft()
                    fn()
                    clk["pe"] += cost
                    if not steps:
                        emitted.add(key)
                        fillers.popleft()
                        open_unit[0] = None
                    else:
                        open_unit[0] = key
                if clk["pe"] < t:
                    clk["starve"] += t - clk["pe"]
                    STARVE_LOG.append((CUR_JOB[0], round(t - clk["pe"]),
                                       [k for k in fillers
                                        if k not in emitted]))

            # PSUM slots pace emission: allocating one waits (in hardware)
            # for its previous tenant's consumer to drain, so model each
            # slot ring's free times and pour filler into score-slot waits.
            # Scores own the "s" ring; filler/proj units share the 2-slot
            # "aux" ring so they never stall the score stream.
            s_free = deque([0.0, 0.0, 0.0, 0.0])
            aux_free = deque([0.0, 0.0])

            def s_gate():
                gate = s_free.popleft()
                fill_until(gate)
                clk["pe"] = max(clk["pe"], gate)

            def aux_gate():
                gate = aux_free.popleft()
                clk["pe"] = max(clk["pe"], gate)

            # ---- filler units as micro-step lists ----
            def v_steps(st):
                cell = {}

                def step(dc):
                    if dc == 0:
                        aux_gate()
                        cell["ps"] = paux.tile([128, FQK], F32, tag="aux", name="ps_v")
                    nc.tensor.matmul(
                        cell["ps"][:, :],
                        xT[:, dc, st * 128:(st + 1) * 128],
                        wqkv_sb[:, 6:9, dc, :],
                        start=(dc == 0), stop=(dc == N_DT - 1))
                    if dc == N_DT - 1:
                        drain_copy(
                            vn[:, st, :, 0:HD],
                            cell["ps"][:, :].rearrange(
                                "p (h d) -> p h d", d=HD), FQK)
                        aux_free.append(clk["pe"] + FQK * PE_C + 800.0)
                return [(FQK * PE_C, lambda d=dc: step(d))
                        for dc in range(N_DT)]

            def ft_steps(ft, sc):
                cell = {}

                def step(dc):
                    if dc == 0:
                        aux_gate()
                        cell["ps"] = paux.tile([128, 512], F32, tag="aux", name="ps_qk")
                    nc.tensor.matmul(
                        cell["ps"][:, :],
                        wqkv_sb[:, ft, dc, :],
                        xT[:, dc, sc * 512:(sc + 1) * 512],
                        start=(dc == 0), stop=(dc == N_DT - 1))
                    if dc == N_DT - 1:
                        drain_bias_add(qkT[:, ft, sc * 512:(sc + 1) * 512],
                                       cell["ps"][:, :],
                                       bqk_sb[:, ft:ft + 1], 512)
                        aux_free.append(clk["pe"] + 512 * PE_C + 800.0)
                return [(512 * PE_C, lambda d=dc: step(d))
                        for dc in range(N_DT)]

            proj_osb = {}

            def proj_steps(st, tail=False):
                """Both 384-col halves of one s-tile's projection as a
                single 6-step unit (order within the unit is guaranteed; the
                oc=1 drain ships the whole row)."""
                cell = {}

                def step(oc, ht):
                    if oc == 0 and ht == 0:
                        proj_osb[st] = pout.tile([128, D], F16,
                                                 tag="osb", name="o_sb")
                    if ht == 0:
                        if tail:
                            ps_big = pp.tile([128, 512], F32, tag="s",
                                             name="ps_big")
                            cell["ps"] = ps_big[:, 0:384]
                        else:
                            aux_gate()
                            cell["ps"] = paux.tile([128, 384], F32,
                                                   tag="aux", name="ps_d")
                    nc.tensor.matmul(
                        cell["ps"][:, :],
                        oT[:, ht, st, :],
                        wout_sb[:, ht, oc * 384:(oc + 1) * 384],
                        start=(ht == 0), stop=(ht == FQK // 128 - 1))
                    if ht == FQK // 128 - 1:
                        o_sb = proj_osb[st]
                        drain_copy(o_sb[:, oc * 384:(oc + 1) * 384],
                                   cell["ps"][:, :], 384)
                        if not tail:
                            aux_free.append(clk["pe"] + 384 * PE_C + 700.0)
                        if oc == 1:
                            nc.sync.dma_start(
                                out_d[st * 128:(st + 1) * 128, :], o_sb[:])
                            clk["hwdge"] = (max(clk["hwdge"], clk["pe"])
                                            + 650.0)
                            del proj_osb[st]

                return [(384 * PE_C, lambda o=oc, t=ht: step(o, t))
                        for oc in range(2) for ht in range(FQK // 128)]

            def emit_proj(st, tail=False):
                for cost, fn in proj_steps(st, tail):
                    fn()
                    clk["pe"] += cost

            for _ft in range(6):
                for _sc in range(4):
                    register(("ft", _ft, _sc), ft_steps(_ft, _sc))
            for _st in range(N_ST):
                register(("v", _st), v_steps(_st))

            # initial queue order: first-use-ish; later heads' weight tiles
            # and V tiles trail so they stay banked for the ACT-heavy midgame
            fillers.clear()
            for key in ([("ft", 0, 0), ("ft", 3, 0), ("v", 0), ("v", 1),
                         ("ft", 0, 1), ("ft", 3, 1), ("v", 2), ("v", 3)]
                        + [("ft", f, s) for s in range(2)
                           for f in (1, 4, 2, 5)]
                        + [("v", _st) for _st in range(4, 10)]
                        + [("ft", f, s) for s in range(2, 4)
                           for f in (0, 3, 1, 4, 2, 5)]
                        + [("v", _st) for _st in range(10, 16)]):
                fillers.append(key)

            def prioritize(key):
                if key in emitted or key not in filler_fns:
                    return
                try:
                    fillers.remove(key)
                except ValueError:
                    pass
                fillers.appendleft(key)

            # ---- attention job: k-loop with a cross-job PV-deferral
            # window: flushes ride a global FIFO, popped W-deep so each
            # k-tile's exp has ~W tile-times to land before PE reads it ----
            FLUSH_W = 5
            flushq = deque()      # (job_idx, flush_closure)
            normed = {qc: 0 for qc in range(N_QC)}

            def note_normed(h, qc, last_block=False):
                normed[qc] += 1
                if h % 2 == 1:
                    # both heads of the pair normalized: transpose the shared
                    # o_pair tile into oT.  Normally via XBAR DMA (off-PE);
                    # for the final block's late pairs the HWDGE round-trip
                    # would sit on the critical tail, so PE-transpose instead.
                    pair = h // 2
                    op_t = opair_store.pop((pair, qc))
                    if last_block and pair >= 1:
                        aux_gate()
                        ps_tr = paux.tile([128, 512], F16, tag="aux",
                                          name="ps_tr")
                        for qt in range(4):
                            for hh in range(2):
                                nc.tensor.transpose(
                                    ps_tr[hh * 64:(hh + 1) * 64,
                                          qt * 128:(qt + 1) * 128],
                                    op_t[:, qt, hh * 64:(hh + 1) * 64],
                                    ident[:])
                        clk["pe"] += 8 * 128 * PE_C
                        for sti in range(4):
                            drain_copy(oT[:, pair, qc * 4 + sti, :],
                                       ps_tr[:, sti * 128:(sti + 1) * 128],
                                       128)
                        aux_free.append(clk["pe"] + 8 * 128 * PE_C + 800.0)
                    else:
                        nc.sync.dma_start_transpose(
                            oT[:, pair, qc * 4:(qc + 1) * 4, :], op_t[:])
                        clk["hwdge"] = max(clk["hwdge"], clk["pe"]) + 650.0
                if normed[qc] == HPC:
                    # proj waits the XBARs through the serial HWDGE queue
                    rdy = max(clk["pe"] + 1500.0, clk["hwdge"] + 1200.0)
                    if last_block:
                        rdy = clk["pe"] + 600.0
                    for st in range(qc * 4, qc * 4 + 4):
                        register(("proj", st), proj_steps(st), ready=rdy)

            def force_deps(h, qc):
                # q f_tile chunk qc; k chunks are forced JIT in the k-loop
                force(("ft", h // 2, qc))
                force(("ft", 3 + h // 2, 0))

            def emit_exp(ps_s, pt, offs, diag_js):
                """Exp one 2-k-tile pair on ACT (true exp) or DVE (fast-exp),
                whichever is modeled-free sooner (GPSIMD cannot read PSUM);
                diagonal tiles get a causal mask on Pool.  Returns the
                flush-ready time."""
                if offs[0] >= 256:
                    spans = [(j * 512 + offs[j], (j + 1) * 512)
                             for j in range(2)]
                else:
                    spans = [(offs[0], 1024)]
                cols = sum(b - a for a, b in spans)
                n = len(spans)
                start = clk["pe"] + 150.0
                fins = {
                    "act": max(clk["act"], start) + cols * ACT_C + n * 370.0,
                    "dve": max(clk["dve"], start) + cols * 1.05 + n * 400.0,
                }
                eng = min(fins, key=lambda e: fins[e])
                if eng == "act":
                    for a, b in spans:
                        nc.scalar.activation(
                            pt[:, a:b], ps_s[:, a:b],
                            mybir.ActivationFunctionType.Exp,
                            bias=expb[:], scale=1.0 / A_EXP)
                else:
                    for a, b in spans:
                        nc.vector.tensor_scalar(
                            pt[:, a:b].bitcast(I16), ps_s[:, a:b],
                            T_EXP, B_EXP,
                            mybir.AluOpType.max, mybir.AluOpType.add)
                clk[eng] = fins[eng]
                clk[eng + "_cols"] += cols
                fin = fins[eng]
                for j in diag_js:
                    # causal mask, emitted right after the exp so Pool
                    # applies it long before the deferred PV flush
                    q_off = offs[j]
                    sl = slice(j * 512 + q_off, j * 512 + q_off + 128)
                    nc.gpsimd.affine_select(
                        out=pt[:, sl], in_=pt[:, sl],
                        compare_op=mybir.AluOpType.is_ge,
                        fill=0.0, base=0, channel_multiplier=-1,
                        pattern=[[1, 128]])
                    clk["pool"] = (max(clk["pool"], fin + 100.0)
                                   + 128 * POOL_C + 100.0)
                    fin = clk["pool"]
                return fin + 150.0

            def emit_exp1(ps_s, pt, q_off, diag):
                cols = 512 - q_off
                start = clk["pe"] + 150.0
                fins = {
                    "act": max(clk["act"], start) + cols * ACT_C + 370.0,
                    "dve": max(clk["dve"], start) + cols * 1.05 + 400.0,
                }
                eng = min(fins, key=lambda e: fins[e])
                if eng == "act":
                    nc.scalar.activation(
                        pt[:, q_off:], ps_s[:, q_off:],
                        mybir.ActivationFunctionType.Exp,
                        bias=expb[:], scale=1.0 / A_EXP)
                else:
                    nc.vector.tensor_scalar(
                        pt[:, q_off:].bitcast(I16), ps_s[:, q_off:],
                        T_EXP, B_EXP,
                        mybir.AluOpType.max, mybir.AluOpType.add)
                clk[eng] = fins[eng]
                clk[eng + "_cols"] += cols
                fin = fins[eng]
                if diag:
                    nc.gpsimd.affine_select(
                        out=pt[:, q_off:q_off + 128],
                        in_=pt[:, q_off:q_off + 128],
                        compare_op=mybir.AluOpType.is_ge,
                        fill=0.0, base=0, channel_multiplier=-1,
                        pattern=[[1, 128]])
                    clk["pool"] = (max(clk["pool"], fin + 100.0)
                                   + 128 * POOL_C + 100.0)
                    fin = clk["pool"]
                return fin + 150.0

            def attn_job(job_idx, h, qc):
                po = (h % 2) * 64
                qf = h // 2
                kf = 3 + h // 2
                # the ps_o ring has 2 slots: job i-2's accumulation (incl.
                # normalize) must fully drain before this job's open
                while flushq and flushq[0][0] <= job_idx - 2:
                    flushq.popleft()[1]()
                force_deps(h, qc)
                ps_o = ppo.tile([128, 4, HD + 1], F32, tag="o")
                n_kt = 4 * (qc + 1)
                # open ONE accumulation group for the whole bank, zeroing all
                # four qt regions; every PV then accumulates with start=False
                nc.tensor.matmul(ps_o[:, :, :], ident[:], zeros[:],
                                 start=True, stop=False)
                clk["pe"] += 4 * (HD + 1) * PE_C

                def flush(kt, pt, base, ready, last):
                    fill_until(ready)
                    clk["pe"] = max(clk["pe"], ready)
                    force(("v", kt))
                    for qt in range(4):
                        if kt <= qc * 4 + qt:
                            nc.tensor.matmul(
                                ps_o[:, qt, :],
                                pt[:, base + qt * 128:base + (qt + 1) * 128],
                                vn[:, kt, h, :],
                                start=False,
                                stop=(kt == n_kt - 1 and qt == 3))
                            clk["pe"] += 65 * PE_C
                    if last:
                        recip = prc.tile([128, 4, 1], F32, tag="rc")
                        nc.vector.reciprocal(recip[:], ps_o[:, :, HD:HD + 1])
                        pair = h // 2
                        if (pair, qc) not in opair_store:
                            opair_store[(pair, qc)] = pop.tile(
                                [128, 4, 128], F16, tag="opair", name="o_pair")
                        op_t = opair_store[(pair, qc)]
                        # one batched normalize: recip broadcast along d
                        nc.vector.tensor_tensor(
                            op_t[:, :, po:po + HD], ps_o[:, :, 0:HD],
                            recip[:].broadcast_to([128, 4, HD]),
                            mybir.AluOpType.mult)
                        dve_work(4 * HD)
                        note_normed(h, qc, last_block=(qc == 0 and h >= 2))

                for kt in range(n_kt):
                    force(("ft", kf, kt // 4))
                    s_gate()
                    ps_s = pp.tile([128, 512], F32, tag="s")
                    pt = ppt.tile([128, 512], F16, tag="pt")
                    q_off = max(0, kt * 128 - qc * 512)
                    diag = kt * 128 >= qc * 512
                    nc.tensor.matmul(
                        ps_s[:, q_off:],
                        qkT[po:po + 64, kf, kt * 128:(kt + 1) * 128],
                        qkT[po:po + 64, qf,
                            qc * 512 + q_off:(qc + 1) * 512],
                        start=True, stop=True)
                    clk["pe"] += (512 - q_off) * PE_C
                    ready = emit_exp1(ps_s, pt, q_off, diag)
                    s_free.append(ready - 150.0)
                    flushq.append(
                        (job_idx, lambda k=kt, p=pt, r=ready,
                         l=(kt == n_kt - 1): flush(k, p, 0, r, l)))
                    while len(flushq) > FLUSH_W:
                        flushq.popleft()[1]()

            for i, (h, qc) in enumerate(JOB_ORDER):
                _mark(nc, f"C:h{h} qc{qc}")
                CUR_JOB[0] = f"h{h}q{qc}"
                if i + 1 < len(JOB_ORDER):
                    # float the next job's dep tiles to the queue front so
                    # they get pulled as filler during THIS job instead of
                    # landing as an ACT-stalling blob at the boundary
                    nh, nqc = JOB_ORDER[i + 1]
                    for key in [("ft", 3 + nh // 2, sc)
                                for sc in range(nqc, -1, -1)] + \
                               [("ft", nh // 2, nqc)]:
                        prioritize(key)
                attn_job(i, h, qc)

            _mark(nc, "D:tail")
            CUR_JOB[0] = "tail"
            while flushq:   # final jobs' deferred PVs + normalizes
                flushq.popleft()[1]()
            if open_unit[0] is not None:
                force(open_unit[0])
            # drain every remaining filler; projs not yet emitted use the
            # (now free) score-psum banks so halves never serialize
            for st in range(N_ST):
                key = ("proj", st)
                if key in filler_fns and key not in emitted:
                    emitted.add(key)
                    emit_proj(st, tail=True)
            # anything else (ft/v) must already be in; assert coverage
            for key in filler_fns:
                assert key in emitted, f"filler {key} never emitted"
            assert not opair_store, f"unfinished o_pair {list(opair_store)}"
            EMIT_STATS.update(clk)


def _build():
    if "nc" not in _CACHE:
        nc = bacc.Bacc("TRN2", target_bir_lowering=False, debug=False,
                       num_devices=N_CORES)
        _emit(nc)
        nc.compile()
        _CACHE["nc"] = nc
    return _CACHE["nc"]


def kernel(x, qkv_w, qkv_b, out_w, out_b):
    x = np.asarray(x, dtype=np.float32)
    qkv_w = np.asarray(qkv_w, dtype=np.float32)
    qkv_b = np.asarray(qkv_b, dtype=np.float32)
    out_w = np.asarray(out_w, dtype=np.float32)
    out_b = np.asarray(out_b, dtype=np.float32)

    nc = _build()
    scale = HD ** -0.5 * A_EXP   # score scale + fast-exp prescale
    in_maps = []
    for c in range(N_CORES):
        b, half = c // 2, c % 2
        fq = slice(half * FQK, (half + 1) * FQK)
        fk = slice(D + half * FQK, D + (half + 1) * FQK)
        fv = slice(2 * D + half * FQK, 2 * D + (half + 1) * FQK)
        wq = qkv_w[:, fq] * scale
        wk = qkv_w[:, fk]
        wv = qkv_w[:, fv]
        wqkv = np.concatenate([wq, wk, wv], axis=1)   # [768, 1152]
        # swizzle to [p, ftile, dtile, f] (one contiguous 1536B run per
        # partition per f-tile DMA)
        wqkv = np.ascontiguousarray(
            wqkv.reshape(N_DT, 128, 9, 128).transpose(1, 2, 0, 3),
            dtype=np.float16)
        bqk = np.concatenate([qkv_b[fq] * scale, qkv_b[fk]])  # [768]
        bqk = np.ascontiguousarray(
            bqk.reshape(6, 128).T, dtype=np.float32)          # [128, 6]
        wout = np.ascontiguousarray(
            out_w[half * FQK:(half + 1) * FQK, :], dtype=np.float16)
        in_maps.append({
            "xt": np.ascontiguousarray(x[b].T).astype(np.float16),
            "wqkv": wqkv, "bqk": bqk, "wout": wout,
        })

    res = run_bass_kernel_spmd(nc, in_maps, list(range(N_CORES)), trace=TRACE)
    parts = [res.results[c]["out"] for c in range(N_CORES)]
    # the V bias shifts every attention output by exactly qkv_b[2D:]
    # (softmax weights sum to 1), so its effect on the final output is the
    # constant row qkv_b[2D:] @ out_w - applied here instead of on-core
    bias_row = out_b + qkv_b[2 * D:] @ out_w
    out = np.empty((B, S, D), dtype=np.float32)
    for b in range(B):
        out[b] = (parts[2 * b].astype(np.float32)
                  + parts[2 * b + 1].astype(np.float32) + bias_row)
    if TRACE:
        kernel.last_results = res
    return out
